# revision 32
# baseline (speedup 1.0000x reference)
"""Trainium2 Bass kernel for nn_FFNet_17600775979626.

Spiking FFN layer: cur = einsum('tbi,oi->tbo', x, W) + b, followed by a
leaky-integrate-and-fire scan over T with subtractive reset (snntorch Leaky,
beta=0.95, threshold=1.0). Returns spk_rec [T, B, NO] (0.0/1.0 floats).

Distribution: output-neuron sharding. Each of the 8 cores computes all
(T, B) for a 256-wide slice of the 2048 output neurons; x is replicated.

GEMM scheme (fp16 main + fp8-DoubleRow corrections, ~1.5x over fp16x2):
  x = xh + xl (fp16 split), W = Wh + Wl (fp16 split). Then
    cur ~= xh*Wh + 2^-16 * [ (xl*2^12)_e4m3 * (Wh*2^4)_e4m3
                           + (x)_e4m3     * (Wl*2^16)_e4m3 ]
  The main pass runs in fp16 (1 cycle/row). Both correction terms share the
  2^16 product scale, so they concatenate along K (4096 total) into ONE
  PSUM accumulation group of fp8e4 DoubleRow matmuls (2 MACs/cell/cycle).
  cur error ~1e-5 rms -> ~240 flipped spikes of a ~1276 budget (rel 2e-2).

Orientation: W is stationary (out = W_chunk.T @ x, PSUM [o=128, t*b]); the
N=512 moving stream (TQ=4 timesteps x B=128) amortizes LDWEIGHTS. Only the
xl fp8 plane ships over DMA; the x fp8 plane is cast from xh on the ACT
engine (one batch ahead so the cast never blocks the descale in the ACT
FIFO). The LIF scan runs in [o, b] layout with a v-carry (v = beta*m - spk;
bias folded into the ACT descale's per-partition bias vector), 4 DVE ops
per step. Spikes go out as fp8 [tq, ot, o, tt, b] (0/1 exact); the host
transposes to [T, B, NO] f32 outside HW time.

PSUM: [128, 2, 512] tiles (2 banks: o-tile major) x {main, corr} x
double-buffer = exactly 8 banks. DMA is split across both HWDGE queues
(SP: xh + spikes, ACT: xc + weights).

Walrus codegen on this target accepts at most ONE sync-wait command per
engine instruction, while Tile's wait assigner freely emits several. Two
post-scheduling passes fix that: _slim_waits drops waits already implied
transitively (per-queue FIFO dispatch + semaphore vector clocks), and
_split_waits moves any excess waits onto injected same-queue NoOps.
"""

import numpy as np

T, B, NI, NO = 128, 128, 2048, 2048
NCORES = 8
O_S = NO // NCORES  # 256 output neurons per core = 2 o-tiles of 128
KC = NI // 128  # 16 fp16 contraction chunks
KC8 = 2 * KC  # 32 fp8 chunks: 0..15 xl-term, 16..31 x-term
TQ = 4  # timesteps per batch -> N = TQ*B = 512 moving columns
NB = T // TQ  # 32 batches
BETA = 0.95
S_PROD = float(2.0**16)  # fp8 correction product scale
S_XL = float(2.0**12)  # xl plane scale (term A)
S_WH = float(2.0**4)  # Wh plane scale (term A); S_XL*S_WH == S_PROD
S_WL = S_PROD  # Wl plane scale (term B; x plane at scale 1)

MODE = "fp16fp8"

_cache = {}


def _build_nc(mode=MODE):
    from contextlib import ExitStack

    import concourse.bass as bass
    import concourse.mybir as mybir
    import concourse.tile as tile

    f32 = mybir.dt.float32
    f16 = mybir.dt.float16
    f8 = mybir.dt.float8e4
    DR = mybir.MatmulPerfMode.DoubleRow
    N = TQ * B

    nc = bass.Bass()
    wh = nc.declare_dram_parameter("wh", [128, KC, O_S], f16, isOutput=False)
    wc = nc.declare_dram_parameter("wc", [128, KC8, O_S], f8, isOutput=False)
    bv = nc.declare_dram_parameter("bv", [128, 2], f32, isOutput=False)
    xh = nc.declare_dram_parameter("xh", [128, KC, T * B], f16, isOutput=False)
    # fp8 planes shipped: xl*2^12 only; the x plane is cast from xh on ACT
    xc = nc.declare_dram_parameter("xc", [128, KC, T * B], f8, isOutput=False)
    # [tq, o-tile, o-in-tile, tt, b] fp8; host transposes to [T, B, O_S]
    spk = nc.declare_dram_parameter(
        "spk", [T // TQ, 2, 128, TQ, B], f8, isOutput=True
    )

    with tile.TileContext(nc) as tc, ExitStack() as ctx:
        singles = ctx.enter_context(tc.tile_pool(name="singles", bufs=1))
        xpool = ctx.enter_context(tc.tile_pool(name="xp", bufs=3))
        spool = ctx.enter_context(tc.tile_pool(name="sp", bufs=3))
        tpool = ctx.enter_context(tc.tile_pool(name="tp", bufs=2))
        psum = ctx.enter_context(tc.tile_pool(name="ps", bufs=2, space="PSUM"))

        # SP queue: x fp16 batches + spikes out; ACT queue: weights + fp8 x.
        wh_sb = singles.tile([128, KC, O_S], f16)
        nc.scalar.dma_start(out=wh_sb[:], in_=wh[:])
        wc_sb = singles.tile([128, KC8, O_S], f8)
        bv_sb = singles.tile([128, 2], f32)

        m_sb = singles.tile([128, 2, B], f32)  # membrane potential
        v_sb = singles.tile([128, 2, B], f32)  # carry: beta*m - spk
        nc.vector.memset(v_sb[:], 0.0)

        spk_r = spk[:].rearrange("tq ot o tt b -> tq o ot tt b")

        xh_ts, xc_ts = {}, {}

        def fetch_xh(tq):
            """DMA batch tq's fp16 x plane (SP queue)."""
            xh_t = xpool.tile([128, KC, N], f16, tag="xh", name=f"xh{tq}")
            nc.sync.dma_start(out=xh_t[:], in_=xh[:, :, tq * N : (tq + 1) * N])
            xh_ts[tq] = xh_t

        def fetch_xc(tq):
            """DMA batch tq's fp8 xl plane; ACT casts the x plane from xh."""
            xc_t = xpool.tile([128, KC8, N], f8, tag="xc", name=f"xc{tq}")
            nc.scalar.dma_start(
                out=xc_t[:, :KC], in_=xc[:, :, tq * N : (tq + 1) * N]
            )
            nc.scalar.activation(
                xc_t[:, KC:], xh_ts[tq][:], mybir.ActivationFunctionType.Identity
            )
            xc_ts[tq] = xc_t

        def fetch(tq):
            fetch_xh(tq)
            fetch_xc(tq)

        fetch(0)
        # wc/bv are not needed until the first DR group / descale; issuing
        # them after batch 0's x keeps the shared DMA device free for the
        # first mains' inputs (wh + xh0).
        nc.scalar.dma_start(out=wc_sb[:], in_=wc[:])
        nc.scalar.dma_start(out=bv_sb[:], in_=bv[:])
        fetch(1)
        for tq in range(NB):
            # SP-side prefetch two ahead at the top: xh(tq+2) queues after
            # st(tq-1) and so fires once scan(tq-1) lands -- still a full
            # batch before its consumer.
            if tq + 2 < NB:
                fetch_xh(tq + 2)
            xh_t, xc_t = xh_ts.pop(tq), xc_ts.pop(tq)

            pm = psum.tile([128, 2, N], f32, tag="m")
            pc = psum.tile([128, 2, N], f32, tag="c")
            last = tq == NB - 1

            def mains_full():
                for ot in range(2):
                    osl = slice(ot * 128, (ot + 1) * 128)
                    for k in range(KC):
                        nc.tensor.matmul(
                            pm[:, ot],
                            lhsT=wh_sb[:, k, osl],
                            rhs=xh_t[:, k, :],
                            start=(k == 0),
                            stop=(k == KC - 1),
                        )

            def drs():
                for ot in range(2):
                    osl = slice(ot * 128, (ot + 1) * 128)
                    for k in range(KC):
                        nc.tensor.matmul(
                            pc[:, ot],
                            lhsT=wc_sb[:, 2 * k : 2 * k + 2, osl],
                            rhs=xc_t[:, 2 * k : 2 * k + 2, :],
                            start=(k == 0),
                            stop=(k == KC - 1),
                            perf_mode=DR,
                        )

            if not last:
                mains_full()
                drs()
            else:
                # Tail trim: DR first so the descale overlaps the mains, and
                # mains in per-timestep N=128 groups (tt-outer) so the scan
                # starts as soon as each step's main accumulation lands.
                drs()
                for tt in range(TQ):
                    bsl = slice(tt * B, (tt + 1) * B)
                    for ot in range(2):
                        osl = slice(ot * 128, (ot + 1) * 128)
                        for k in range(KC):
                            nc.tensor.matmul(
                                pm[:, ot, bsl],
                                lhsT=wh_sb[:, k, osl],
                                rhs=xh_t[:, k, bsl],
                                start=(k == 0),
                                stop=(k == KC - 1),
                            )

            # Descale the correction bank on the ACT engine (one scaled
            # PSUM->SBUF copy per o-tile, folding in the bias as the ACT
            # per-partition bias vector), so each DVE op below reads at most
            # one PSUM operand (walrus ISA rule).
            curc = tpool.tile([128, 2, N], f32, tag="curc")
            for ot in range(2):
                nc.scalar.activation(
                    curc[:, ot],
                    pc[:, ot],
                    mybir.ActivationFunctionType.Identity,
                    bias=bv_sb[:, ot : ot + 1],
                    scale=1.0 / S_PROD,
                )

            # ACT-side prefetch emitted AFTER the descale: the descale must
            # never queue behind a cast that waits on a late xh load (the
            # spike store st(tq-1) gates xh(tq+2) in the SP FIFO), or the
            # store->load->cast->descale->scan->store loop couples into a
            # period longer than one batch. In this order every ACT entry
            # waits only on batch-locked events, with >= a batch of slack.
            if tq + 2 < NB:
                fetch_xc(tq + 2)

            st = spool.tile([128, 2, TQ, B], f8)
            for tt in range(TQ):
                bsl = slice(tt * B, (tt + 1) * B)
                cur = tpool.tile([128, 2, B], f32, tag="cur")
                nc.vector.tensor_tensor(
                    cur[:], curc[:, :, bsl], pm[:, :, bsl], mybir.AluOpType.add
                )
                nc.vector.tensor_tensor(
                    m_sb[:], v_sb[:], cur[:], mybir.AluOpType.add
                )
                nc.vector.tensor_scalar(
                    st[:, :, tt], m_sb[:], 1.0, None, mybir.AluOpType.is_gt
                )
                nc.vector.scalar_tensor_tensor(
                    v_sb[:],
                    m_sb[:],
                    BETA,
                    st[:, :, tt],
                    mybir.AluOpType.mult,
                    mybir.AluOpType.subtract,
                )
            nc.sync.dma_start(out=spk_r[tq], in_=st[:])

    _slim_waits(nc)
    _split_waits(nc)
    return nc


def _slim_waits(nc):
    """Drop sync waits already implied by earlier ones (transitive closure).

    Each engine queue dispatches in FIFO order, so a wait satisfied on an
    earlier instruction of the same queue covers later instructions. A wait
    on sem s >= v also imports everything the incrementing instruction's
    queue had itself waited for when it raised s to v (semaphore vector
    clocks with snapshots at each increment).
    """
    FRAMEWORK_OPS = ("InstEventSemaphore", "InstDrain")
    engine_clock = {}  # engine -> {sem_id: value known reached}
    totals = {}  # sem_id -> running total of increments
    snapshots = {}  # sem_id -> [(value, clock dict)] in increasing value order
    poisoned = set()  # sems touched by non-monotonic updates (barriers)

    def join(dst, src):
        for s, v in src.items():
            if s in poisoned:
                continue
            if dst.get(s, -1) < v:
                dst[s] = v

    for blk in nc.m.functions[0].blocks:
        for inst in blk.instructions:
            si = getattr(inst, "sync_info", None)
            if si is None:
                continue
            is_framework = type(inst).__name__ in FRAMEWORK_OPS
            clock = engine_clock.setdefault(inst.engine, {})
            if si.on_wait:
                kept = []
                for w in si.on_wait:
                    if (
                        w.sync_type != "semaphore"
                        or w.wait_mode != "sem-ge-imm"
                        or w.id in poisoned
                    ):
                        kept.append(w)
                        continue
                    covered = clock.get(w.id, -1) >= w.wait_value
                    for val, snap in snapshots.get(w.id, ()):
                        if val <= w.wait_value:
                            join(clock, snap)
                        else:
                            break
                    if clock.get(w.id, -1) < w.wait_value:
                        clock[w.id] = w.wait_value
                    if is_framework or not covered:
                        kept.append(w)
                si.on_wait = kept
            if si.on_update:
                for u in si.on_update:
                    if u.sync_type != "semaphore":
                        continue
                    if u.update_mode not in ("sem-inc", "sem-add-imm"):
                        # barrier-style sem: stop reasoning about it entirely
                        poisoned.add(u.id)
                        totals.pop(u.id, None)
                        snapshots.pop(u.id, None)
                        for c in engine_clock.values():
                            c.pop(u.id, None)
                        continue
                    if u.id in poisoned:
                        continue
                    tot = totals.get(u.id, 0) + (u.update_value or 1)
                    totals[u.id] = tot
                    snap = dict(clock)
                    snap[u.id] = tot
                    snapshots.setdefault(u.id, []).append((tot, snap))


def _split_waits(nc, limit=1):
    """Move excess sync waits onto injected same-queue NoOps.

    Walrus codegen accepts at most `limit` sync-wait commands per engine
    instruction on this target. Engine queues dispatch in order, so a
    preceding NoOp carrying the wait is equivalent.
    """
    import concourse.mybir as mybir

    n_nops = 0
    for blk in nc.m.functions[0].blocks:
        out = []
        changed = False
        for inst in blk.instructions:
            si = getattr(inst, "sync_info", None)
            if type(inst).__name__ == "InstEventSemaphore":
                out.append(inst)
                continue
            if si is not None and si.on_wait and len(si.on_wait) > limit:
                waits = list(si.on_wait)
                for w in waits[:-limit]:
                    nop = mybir.InstNoOp(name=f"wnop-{n_nops}", ins=[], outs=[])
                    n_nops += 1
                    nop.engine = inst.engine
                    nop.sync_info = mybir.SyncInfo(on_wait=[w], on_update=[])
                    nop.bass_nofuse = True
                    out.append(nop)
                    changed = True
                si.on_wait = waits[-limit:]
            out.append(inst)
        if changed:
            try:
                blk.instructions = out
            except Exception:
                blk.instructions.clear()
                blk.instructions.extend(out)


def _to_pk(a, nchunks):
    """[rows=nchunks*128, cols] -> [128, nchunks, cols] (p-major chunks)."""
    r, c = a.shape
    return np.ascontiguousarray(a.reshape(nchunks, 128, c).transpose(1, 0, 2))


def _prepare_in_maps(x, W, b):
    import ml_dtypes

    e4 = ml_dtypes.float8_e4m3

    x = np.ascontiguousarray(x, dtype=np.float32)
    W = np.ascontiguousarray(W, dtype=np.float32)
    b = np.ascontiguousarray(b, dtype=np.float32)

    x2 = x.reshape(T * B, NI)
    xh16 = x2.astype(np.float16)
    xl = x2 - xh16.astype(np.float32)
    xh_d = _to_pk(np.ascontiguousarray(xh16.T), KC)
    xc_d = _to_pk(np.ascontiguousarray((xl * S_XL).astype(e4).T), KC)

    WT = np.ascontiguousarray(W.T)  # [NI, NO]
    in_maps = []
    for c in range(NCORES):
        Wc = WT[:, c * O_S : (c + 1) * O_S]
        Wh = Wc.astype(np.float16)
        Wl = Wc - Wh.astype(np.float32)
        wh_d = _to_pk(Wh, KC)
        wc_d = np.ascontiguousarray(
            np.concatenate(
                [_to_pk((Wh.astype(np.float32) * S_WH).astype(e4), KC),
                 _to_pk((Wl * S_WL).astype(e4), KC)],
                axis=1,
            )
        )
        bv_d = np.ascontiguousarray(
            b[c * O_S : (c + 1) * O_S].reshape(2, 128).T
        )
        in_maps.append(
            {"wh": wh_d, "wc": wc_d, "bv": bv_d, "xh": xh_d, "xc": xc_d}
        )
    return in_maps


def run(x, W, b, trace=False):
    """Run the kernel; returns (out [T,B,NO] fp32, BassKernelResults)."""
    from concourse.bass_utils import run_bass_kernel_spmd

    if MODE not in _cache:
        _cache[MODE] = _build_nc(MODE)
    nc = _cache[MODE]
    in_maps = _prepare_in_maps(x, W, b)
    res = run_bass_kernel_spmd(nc, in_maps, list(range(NCORES)), trace=trace)
    # per-core spk [T//TQ, 2, 128, TQ, B] f16 -> [T, B, 256] f32
    parts = [
        np.ascontiguousarray(res.results[c]["spk"])
        .astype(np.float32)
        .transpose(0, 3, 4, 1, 2)
        .reshape(T, B, O_S)
        for c in range(NCORES)
    ]
    out = np.concatenate(parts, axis=2)
    return out, res


def kernel(x, W, b):
    out, _ = run(x, W, b, trace=False)
    return out
